# revision 2
# baseline (speedup 1.0000x reference)
"""Trainium2 Bass kernel for single-head attention with input projections.

    query = q @ Wq + bq ; key = k @ Wk + bk ; value = v @ Wv + bv
    out   = softmax(query @ key.T / sqrt(H)) @ value
    (q, k, v: [4096, 1024] fp32; Wq/Wk/Wv: [1024, 1024]; out: [4096, 1024])

Runs on 8 NeuronCores (SPMD via run_bass_kernel_spmd), q-rows sharded
512/core, k/v replicated as bf16 host streams (layout/dtype transforms
only, zero host FLOPs), no collectives.

v2 design notes (PE is clock-capped ~2.0 GHz with all 8 cores busy, so
the whole game is MM count and zero PE idle):
  - P0: uT = (q@Wq)^T, tT = (u@Wk^T)^T  (128 MMs; bk cancels in softmax)
  - C0: scoresT chunks = kT-tiles @ tT -> Exp (scale=1/sqrt(H)) -> expT
    (256 MMs).  Row sums accumulate OFF the PE: DVE adds acc += expT
    chunk, then ONE f32 ones-matmul replicates the partition-sum into
    sums_rep [128,512] (every partition = the rowsum vector).
  - CV: cvT[d,sq] accumulated directly with v-chunks as stationary and
    expT as moving (256 MMs, db-outer so each bank's drain hides under
    the next bank's accumulation).  The drain itself multiplies by
    1/rowsum (DVE tensor_tensor with inv_bc, replicated layout) -- no
    PE transposes anywhere.
  - PROJ: out = cvn @ Wv (64 MMs, f32r), + bv if nonzero, DMA out.
DMA: qt/wq issue first (fast first-MM), wkt during uT, kT/v during C0,
Wv during CV.  All phase boundaries have sub-us PE bubbles only.

Precision: bf16 streams + fp32 PSUM accumulation, f32 row sums,
f32r cvT/Wv projection.  Softmax skips max-subtraction (valid while
scaled scores < ~85; reference distribution peaks ~5.5).
"""
import numpy as np

import concourse.bacc as bacc
import concourse.mybir as mybir
import concourse.tile as tile
from concourse import bass_isa
from concourse.bass_utils import run_bass_kernel_spmd

F32 = mybir.dt.float32
F32R = mybir.dt.float32r
BF16 = mybir.dt.bfloat16
AF = mybir.ActivationFunctionType
ALU = mybir.AluOpType

S = 4096
D = 1024
H = 1024
NCORES = 8
SQ = S // NCORES          # 512 q-rows per core
CH = 512                  # k/v streaming chunk (s-rows)
NCH = S // CH             # 8 chunks
NT = D // 128             # 8
NJ = H // 128             # 8
NI = S // 128             # 32 idx blocks (128 s-rows each)
INV_SQRT_H = 1.0 / np.sqrt(np.float32(H))


def build_program(apply_bq: bool, apply_bv: bool):
    nc = bacc.Bacc("TRN2", target_bir_lowering=False, debug=False,
                   enable_asserts=False, num_devices=NCORES)

    qt = nc.dram_tensor("qt", [D, SQ], BF16, kind="ExternalInput").ap()
    ktf = nc.dram_tensor("ktf", [D, S], BF16, kind="ExternalInput").ap()
    vf = nc.dram_tensor("vf", [S, D], BF16, kind="ExternalInput").ap()
    wq = nc.dram_tensor("wq", [D, H], BF16, kind="ExternalInput").ap()
    wkt = nc.dram_tensor("wkt", [H, D], BF16, kind="ExternalInput").ap()
    wv = nc.dram_tensor("wv", [D, H], F32R, kind="ExternalInput").ap()
    ones_d = nc.dram_tensor("ones_d", [128, 128], F32, kind="ExternalInput").ap()
    ones_bb = nc.dram_tensor("ones_bb", [128, 128], BF16, kind="ExternalInput").ap()
    bq_r = nc.dram_tensor("bq_r", [NJ, 128], F32, kind="ExternalInput").ap()
    bv_d = nc.dram_tensor("bv_d", [1, H], F32, kind="ExternalInput").ap()
    ones_f = nc.dram_tensor("ones_f", [1, 128], F32, kind="ExternalInput").ap()
    out = nc.dram_tensor("out", [SQ, H], F32, kind="ExternalOutput").ap()

    with tile.TileContext(nc) as tc:
        with tc.tile_pool(name="persist", bufs=1) as pp:
            tT = pp.tile([128, NT, SQ], BF16)
            expT = pp.tile([128, NI, SQ], BF16)
            v_sb = pp.tile([128, NI, D], BF16)
            cvT = pp.tile([128, NT, SQ], F32R)
            acc = pp.tile([128, SQ], F32)      # per-partition partial rowsums
            inv_bc = pp.tile([128, SQ], F32)   # 1/rowsum, replicated on partitions

            kt_ctx = tc.tile_pool(name="kt_dbl", bufs=2)
            kt_pool = kt_ctx.__enter__()

            # ---- P0: uT then tT ----
            with tc.tile_pool(name="p0", bufs=1) as p0:
                wq_sb = p0.tile([128, NT, H], BF16)
                qt_sb = p0.tile([128, NT, SQ], BF16)
                wkt_sb = p0.tile([128, NJ, D], BF16)
                uT = p0.tile([128, NJ, SQ], BF16)
                ones_bb_sb = p0.tile([128, 128], BF16)
                # The head is DMA-bandwidth bound: sequence the critical 6MB on
                # ONE queue (SP) in exact consumption order.  The ACT queue is
                # reserved for the later v/wv streams so nothing competes here.
                nc.sync.dma_start(ones_bb_sb[:], ones_bb[:])
                for t in range(NT):
                    ts_ = slice(128 * t, 128 * (t + 1))
                    nc.sync.dma_start(qt_sb[:, t, :], qt[ts_, :])
                    nc.sync.dma_start(wq_sb[:, t, :], wq[ts_, :])
                for m in range(NJ):
                    nc.sync.dma_start(wkt_sb[:, m, :], wkt[128 * m:128 * (m + 1), :])
                kt_ch0 = kt_pool.tile([128, NT, CH], BF16, tag="kt")
                for t in range(NT):
                    nc.sync.dma_start(kt_ch0[:, t, :], ktf[128 * t:128 * (t + 1), 0:CH])

                # HAM warm-up: short dummy matmuls while qt/wq stream in
                with tc.tile_pool(name="warm_ps", bufs=1, space="PSUM") as warm_ps:
                    warm = warm_ps.tile([128, 64], F32)
                    for i in range(38):
                        nc.tensor.matmul(warm[:], ones_bb_sb[:], ones_bb_sb[:, 0:64],
                                         start=True, stop=True)
                p0_ps_ctx = tc.tile_pool(name="p0_ps", bufs=8, space="PSUM")
                p0_ps = p0_ps_ctx.__enter__()
                if apply_bq:
                    bq_sb = pp.tile([128, NJ], F32)
                    nc.sync.dma_start(bq_sb[:], bq_r.rearrange("t p -> p t"))
                if apply_bv:
                    bv_row = pp.tile([1, H], F32)
                    nc.sync.dma_start(bv_row[:], bv_d[:])
                    onef = pp.tile([1, 128], F32)
                    nc.sync.dma_start(onef[:], ones_f[:])

                # uT in two j-half passes: the first half's drains overlap the
                # second half's matmuls, so tT is never drain-gated.
                ups = [p0_ps.tile([128, SQ], F32, name=f"ups{j}", tag="ups", bufs=8)
                       for j in range(NJ)]

                def drain_u(j):
                    if apply_bq:
                        if j % 2 == 0:
                            nc.scalar.activation(uT[:, j, :], ups[j][:], AF.Identity,
                                                 bias=bq_sb[:, j:j + 1])
                        else:
                            nc.vector.tensor_scalar_add(uT[:, j, :], ups[j][:],
                                                        bq_sb[:, j:j + 1])
                    else:
                        if j % 2 == 0:
                            nc.scalar.activation(uT[:, j, :], ups[j][:], AF.Copy)
                        else:
                            nc.vector.tensor_copy(uT[:, j, :], ups[j][:])

                for t in range(NT):
                    for j in range(NJ):
                        nc.tensor.matmul(ups[j][:], wq_sb[:, t, 128 * j:128 * (j + 1)],
                                         qt_sb[:, t, :], start=(t == 0),
                                         stop=(t == NT - 1))
                for j in range(NJ):
                    drain_u(j)

                for j2 in range(NT):
                    ps = p0_ps.tile([128, SQ], F32, tag="ups", bufs=8)
                    for m in range(NJ):
                        nc.tensor.matmul(ps[:], wkt_sb[:, m, 128 * j2:128 * (j2 + 1)],
                                         uT[:, m, :], start=(m == 0), stop=(m == NJ - 1))
                    if j2 % 2 == 0:
                        nc.scalar.activation(tT[:, j2, :], ps[:], AF.Copy)
                    else:
                        nc.vector.tensor_copy(tT[:, j2, :], ps[:])
                p0_ps_ctx.__exit__(None, None, None)

            # ---- C0: scoresT -> exp -> expT; DVE accumulates rowsum partials ----
            with (
                tc.tile_pool(name="sc_ps", bufs=3, space="PSUM") as sc_ps,
            ):
                kt_tiles = {0: kt_ch0}
                for c in range(NCH):
                    kt_ch = kt_tiles.pop(c)
                    # prefetch NEXT kt chunk before this chunk's v stream so
                    # the (single) DMA queue never starves the scores MMs
                    if c + 1 < NCH:
                        nxt = kt_pool.tile([128, NT, CH], BF16, tag="kt")
                        for t in range(NT):
                            nc.sync.dma_start(
                                nxt[:, t, :],
                                ktf[128 * t:128 * (t + 1), CH * (c + 1):CH * (c + 2)])
                        kt_tiles[c + 1] = nxt
                    # v rides the SP queue AFTER kt[c+1]: FIFO keeps the head
                    # and the scores stream fed; v itself is only needed at CV.
                    # (On the ACT queue it would be dep-free and the scheduler
                    # hoists it to kernel start, starving the critical head.)
                    nc.sync.dma_start(
                        v_sb[:, 4 * c:4 * (c + 1), :],
                        vf[CH * c:CH * (c + 1), :].rearrange("(u p) d -> p u d", p=128))
                    for u in range(CH // 128):
                        idx = (CH // 128) * c + u
                        ps = sc_ps.tile([128, SQ], F32, tag="sps", bufs=3)
                        for t in range(NT):
                            nc.tensor.matmul(ps[:], kt_ch[:, t, 128 * u:128 * (u + 1)],
                                             tT[:, t, :], start=(t == 0), stop=(t == NT - 1))
                        nc.scalar.activation(expT[:, idx, :], ps[:], AF.Exp,
                                             scale=float(INV_SQRT_H))
                        if idx == 0:
                            nc.vector.tensor_copy(acc[:], expT[:, 0, :])
                        else:
                            nc.vector.tensor_tensor(acc[:], acc[:], expT[:, idx, :],
                                                    op=ALU.add)
            kt_ctx.__exit__(None, None, None)

            # ---- CV: cvT = (w @ v)^T accumulated directly; drain = normalize ----
            with (
                tc.tile_pool(name="cv_ps", bufs=3, space="PSUM") as cv_ps_pool,
                tc.tile_pool(name="wv_pool", bufs=1) as wv_pool,
                tc.tile_pool(name="pj_ps", bufs=2, space="PSUM") as pj_ps,
                tc.tile_pool(name="out_pool", bufs=2) as out_pool,
            ):
                wv_sb = wv_pool.tile([128, NT, H], F32R)
                for t in range(NT):
                    nc.sync.dma_start(wv_sb[:, t, :], wv[128 * t:128 * (t + 1), :])
                if apply_bv:
                    bv_bcast = pp.tile([128, H], F32)
                    with tc.tile_pool(name="bv_ps", bufs=2, space="PSUM") as bv_ps:
                        for half in range(2):
                            hs = slice(512 * half, 512 * (half + 1))
                            psb = bv_ps.tile([128, 512], F32)
                            nc.tensor.matmul(psb[:], onef[:], bv_row[0:1, hs],
                                             start=True, stop=True)
                            nc.scalar.activation(bv_bcast[:, hs], psb[:], AF.Copy)

                # rowsum partition-reduce entirely OFF the PE: GpSimd all-reduce
                # replicates sum_p acc[p, sq] across partitions (in-place),
                # then DVE takes the reciprocal.
                nc.gpsimd.partition_all_reduce(acc[:], acc[:], channels=128,
                                               reduce_op=bass_isa.ReduceOp.add)
                nc.vector.reciprocal(inv_bc[:], acc[:])
                for db in range(NT):
                    cvp = cv_ps_pool.tile([128, SQ], F32, tag="cvp", bufs=3)
                    for idx in range(NI):
                        nc.tensor.matmul(cvp[:], v_sb[:, idx, 128 * db:128 * (db + 1)],
                                         expT[:, idx, :], start=(idx == 0),
                                         stop=(idx == NI - 1))
                    # normalizing drain (PSUM f32 * inv -> SBUF f32r)
                    nc.vector.tensor_tensor(cvT[:, db, :], cvp[:], inv_bc[:],
                                            op=ALU.mult)

                # ---- PROJ: out = cvn @ Wv (+ bv) ----
                for b in range(SQ // 128):
                    for h_ in range(2):
                        hs = slice(512 * h_, 512 * (h_ + 1))
                        last = (b == SQ // 128 - 1 and h_ == 1)
                        if not last:
                            ps = pj_ps.tile([128, 512], F32, tag="ctx")
                            for t in range(NT):
                                nc.tensor.matmul(ps[:], cvT[:, t, 128 * b:128 * (b + 1)],
                                                 wv_sb[:, t, hs], start=(t == 0),
                                                 stop=(t == NT - 1))
                            out_t = out_pool.tile([128, 512], F32, tag="out")
                            if apply_bv:
                                nc.vector.tensor_tensor(out_t[:], ps[:], bv_bcast[:, hs],
                                                        op=ALU.add)
                            else:
                                nc.scalar.activation(out_t[:], ps[:], AF.Copy)
                            nc.sync.dma_start(out[128 * b:128 * (b + 1), hs], out_t[:])
                        else:
                            # final tile: two half-N banks so the two drains +
                            # DMAs run in parallel on ACT and DVE
                            out_t = out_pool.tile([128, 512], F32, tag="out")
                            for qh in range(2):
                                qcol = slice(512 * h_ + 256 * qh,
                                             512 * h_ + 256 * (qh + 1))
                                qs = slice(256 * qh, 256 * (qh + 1))
                                psq = pj_ps.tile([128, 256], F32, tag=f"ctxl{qh}",
                                                 bufs=1)
                                for t in range(NT):
                                    nc.tensor.matmul(
                                        psq[:], cvT[:, t, 128 * b:128 * (b + 1)],
                                        wv_sb[:, t, qcol], start=(t == 0),
                                        stop=(t == NT - 1))
                                if apply_bv:
                                    nc.vector.tensor_tensor(out_t[:, qs], psq[:],
                                                            bv_bcast[:, qcol],
                                                            op=ALU.add)
                                    nc.sync.dma_start(
                                        out[128 * b:128 * (b + 1), qcol], out_t[:, qs])
                                elif qh == 0:
                                    nc.scalar.activation(out_t[:, qs], psq[:], AF.Copy)
                                    nc.scalar.dma_start(
                                        out[128 * b:128 * (b + 1), qcol], out_t[:, qs])
                                else:
                                    nc.vector.tensor_copy(out_t[:, qs], psq[:])
                                    nc.sync.dma_start(
                                        out[128 * b:128 * (b + 1), qcol], out_t[:, qs])

    nc.compile()
    return nc


_CACHE = {}


def _get_program(apply_bq: bool, apply_bv: bool):
    key = (apply_bq, apply_bv)
    if key not in _CACHE:
        _CACHE[key] = build_program(apply_bq, apply_bv)
    return _CACHE[key]


def _prepare_in_maps(ins: dict) -> list:
    import ml_dtypes
    q = np.asarray(ins["q"], np.float32)
    k = np.asarray(ins["k"], np.float32)
    v = np.asarray(ins["v"], np.float32)
    assert q.shape == (S, D) and k.shape == (S, D) and v.shape == (S, D)

    qT = np.ascontiguousarray(q.T).astype(ml_dtypes.bfloat16)
    kT_bf = np.ascontiguousarray(k.T).astype(ml_dtypes.bfloat16)
    v_bf = v.astype(ml_dtypes.bfloat16)
    Wq = np.ascontiguousarray(np.asarray(ins["Wq"], np.float32)).astype(ml_dtypes.bfloat16)
    WkT = np.ascontiguousarray(np.asarray(ins["Wk"], np.float32).T).astype(ml_dtypes.bfloat16)
    Wv = np.ascontiguousarray(np.asarray(ins["Wv"], np.float32))
    bq = np.asarray(ins["bq"], np.float32).reshape(H)
    bv = np.asarray(ins["bv"], np.float32).reshape(H)

    bq_r = np.ascontiguousarray(bq.reshape(NJ, 128))
    bv_d = np.ascontiguousarray(bv.reshape(1, H))

    in_maps = []
    for i in range(NCORES):
        sl = slice(SQ * i, SQ * (i + 1))
        in_maps.append({
            "qt": np.ascontiguousarray(qT[:, sl]),
            "ktf": kT_bf, "vf": v_bf,
            "wq": Wq, "wkt": WkT, "wv": Wv,
            "ones_d": np.ones((128, 128), np.float32),
            "ones_bb": np.ones((128, 128), ml_dtypes.bfloat16),
            "bq_r": bq_r, "bv_d": bv_d,
            "ones_f": np.ones((1, 128), np.float32),
        })
    return in_maps


def kernel(q, k, v, Wq, bq, Wk, bk, Wv, bv) -> np.ndarray:
    # bk shifts all scores in a row uniformly and cancels in softmax.
    ins = {"q": q, "k": k, "v": v, "Wq": Wq, "bq": bq, "Wk": Wk,
           "Wv": Wv, "bv": bv}
    apply_bq = bool(np.any(np.asarray(bq)))
    apply_bv = bool(np.any(np.asarray(bv)))
    nc = _get_program(apply_bq, apply_bv)
    in_maps = _prepare_in_maps(ins)
    res = run_bass_kernel_spmd(nc, in_maps, core_ids=list(range(NCORES)))
    return np.concatenate([res.results[i]["out"] for i in range(NCORES)], axis=0)


# revision 4
# speedup vs baseline: 1.1889x; 1.1889x over previous
"""Trainium2 Bass kernel for single-head attention with input projections.

    query = q @ Wq + bq ; key = k @ Wk + bk ; value = v @ Wv + bv
    out   = softmax(query @ key.T / sqrt(H)) @ value
    (q, k, v: [4096, 1024] fp32; Wq/Wk/Wv: [1024, 1024]; out: [4096, 1024])

Runs on 8 NeuronCores (SPMD via run_bass_kernel_spmd), q-rows sharded
512/core, k/v replicated as bf16 host streams (layout/dtype transforms
only, zero host FLOPs), no collectives.

Design (PE runs 216 ns per 512-free bf16 MM at 2.4 GHz, or 259 ns when
the chip power-throttles to ~2.0 GHz; either way the whole game is MM
count and zero PE idle — this version measures ~3 us total PE idle):
  - 38 N=64 dummy matmuls on a ones tile warm the HAM clock gate while
    the first operands stream in.
  - P0: uT = (q@Wq)^T, tT = (u@Wk^T)^T  (128 MMs; bk cancels in
    softmax; PSUM drains alternate ACT/DVE).
  - C0: scoresT chunks = kT-tiles @ tT -> Exp (scale=1/sqrt(H)) -> expT
    (256 MMs).  Row sums accumulate OFF the PE: DVE adds acc += expT
    chunk, then one GpSimd partition_all_reduce replicates the rowsum
    across partitions (zero PE cost).
  - CV: cvT[d,sq] accumulated directly with v-chunks as stationary and
    expT as moving (256 MMs, db-outer so each bank's drain hides under
    the next bank's accumulation).  The drain itself multiplies by
    1/rowsum (DVE tensor_tensor with the replicated inv layout) -- no
    PE transposes anywhere.
  - PROJ: out = cvn @ Wv (64 MMs, f32r); the last tile is split into
    two half-N PSUM banks so the final drains+DMAs run ACT||DVE.
Scheduling notes (hard-won): the head is DMA-bound, so the critical
6 MB (ones, qt/wq per-t, wkt per-m, kT chunk 0) is sequenced on the SP
queue in exact consumption order; later kt chunks ride SP as ONE
rearranged transfer each, always issued BEFORE that iteration's v
chunk.  Dep-free DMAs on other engine queues get hoisted to kernel
start by the Tile scheduler and would starve the head (do NOT move the
v stream to the ACT queue).  The cv PSUM pool opens before C0 so its
banks are disjoint from the score banks (avoids a WAR stall on the
last exp at the C0->CV boundary).

Precision: bf16 streams + fp32 PSUM accumulation, f32 row sums,
f32r cvT/Wv projection.  Softmax skips max-subtraction (valid while
scaled scores < ~85; reference distribution peaks ~5.5).
"""
import numpy as np

import concourse.bacc as bacc
import concourse.mybir as mybir
import concourse.tile as tile
from concourse import bass_isa
from concourse.bass_utils import run_bass_kernel_spmd

F32 = mybir.dt.float32
F32R = mybir.dt.float32r
BF16 = mybir.dt.bfloat16
AF = mybir.ActivationFunctionType
ALU = mybir.AluOpType

S = 4096
D = 1024
H = 1024
NCORES = 8
SQ = S // NCORES          # 512 q-rows per core
CH = 512                  # k/v streaming chunk (s-rows)
NCH = S // CH             # 8 chunks
NT = D // 128             # 8
NJ = H // 128             # 8
NI = S // 128             # 32 idx blocks (128 s-rows each)
INV_SQRT_H = 1.0 / np.sqrt(np.float32(H))


def build_program(apply_bq: bool, apply_bv: bool):
    nc = bacc.Bacc("TRN2", target_bir_lowering=False, debug=False,
                   enable_asserts=False, num_devices=NCORES)

    qt = nc.dram_tensor("qt", [D, SQ], BF16, kind="ExternalInput").ap()
    ktf = nc.dram_tensor("ktf", [D, S], BF16, kind="ExternalInput").ap()
    vf = nc.dram_tensor("vf", [S, D], BF16, kind="ExternalInput").ap()
    wq = nc.dram_tensor("wq", [D, H], BF16, kind="ExternalInput").ap()
    wkt = nc.dram_tensor("wkt", [H, D], BF16, kind="ExternalInput").ap()
    wv = nc.dram_tensor("wv", [D, H], F32R, kind="ExternalInput").ap()
    ones_d = nc.dram_tensor("ones_d", [128, 128], F32, kind="ExternalInput").ap()
    ones_bb = nc.dram_tensor("ones_bb", [128, 128], BF16, kind="ExternalInput").ap()
    bq_r = nc.dram_tensor("bq_r", [NJ, 128], F32, kind="ExternalInput").ap()
    bv_d = nc.dram_tensor("bv_d", [1, H], F32, kind="ExternalInput").ap()
    ones_f = nc.dram_tensor("ones_f", [1, 128], F32, kind="ExternalInput").ap()
    out = nc.dram_tensor("out", [SQ, H], F32, kind="ExternalOutput").ap()

    with tile.TileContext(nc) as tc:
        with tc.tile_pool(name="persist", bufs=1) as pp:
            tT = pp.tile([128, NT, SQ], BF16)
            expT = pp.tile([128, NI, SQ], BF16)
            v_sb = pp.tile([128, NI, D], BF16)
            cvT = pp.tile([128, NT, SQ], F32R)
            acc = pp.tile([128, SQ], F32)      # per-partition partial rowsums
            inv_bc = pp.tile([128, SQ], F32)   # 1/rowsum, replicated on partitions

            kt_ctx = tc.tile_pool(name="kt_dbl", bufs=2)
            kt_pool = kt_ctx.__enter__()

            # ---- P0: uT then tT ----
            with tc.tile_pool(name="p0", bufs=1) as p0:
                wq_sb = p0.tile([128, NT, H], BF16)
                qt_sb = p0.tile([128, NT, SQ], BF16)
                wkt_sb = p0.tile([128, NJ, D], BF16)
                uT = p0.tile([128, NJ, SQ], BF16)
                ones_bb_sb = p0.tile([128, 128], BF16)
                # The head is DMA-bandwidth bound: sequence the critical 6MB on
                # ONE queue (SP) in exact consumption order.  The ACT queue is
                # reserved for the later v/wv streams so nothing competes here.
                nc.sync.dma_start(ones_bb_sb[:], ones_bb[:])
                for t in range(NT):
                    ts_ = slice(128 * t, 128 * (t + 1))
                    nc.sync.dma_start(qt_sb[:, t, :], qt[ts_, :])
                    nc.sync.dma_start(wq_sb[:, t, :], wq[ts_, :])
                # wkt per-m so the first tT groups start on early blocks;
                # kt chunks as ONE rearranged transfer (1/8th SP issue time)
                for m in range(NJ):
                    nc.sync.dma_start(wkt_sb[:, m, :], wkt[128 * m:128 * (m + 1), :])
                kt_ch0 = kt_pool.tile([128, NT, CH], BF16, tag="kt")
                nc.sync.dma_start(
                    kt_ch0[:],
                    ktf[:, 0:CH].rearrange("(t p) s -> p t s", p=128))

                # HAM warm-up: short dummy matmuls while qt/wq stream in
                with tc.tile_pool(name="warm_ps", bufs=1, space="PSUM") as warm_ps:
                    warm = warm_ps.tile([128, 64], F32)
                    for i in range(38):
                        nc.tensor.matmul(warm[:], ones_bb_sb[:], ones_bb_sb[:, 0:64],
                                         start=True, stop=True)
                p0_ps_ctx = tc.tile_pool(name="p0_ps", bufs=8, space="PSUM")
                p0_ps = p0_ps_ctx.__enter__()
                if apply_bq:
                    bq_sb = pp.tile([128, NJ], F32)
                    nc.sync.dma_start(bq_sb[:], bq_r.rearrange("t p -> p t"))
                if apply_bv:
                    bv_row = pp.tile([1, H], F32)
                    nc.sync.dma_start(bv_row[:], bv_d[:])
                    onef = pp.tile([1, 128], F32)
                    nc.sync.dma_start(onef[:], ones_f[:])

                # uT in two j-half passes: the first half's drains overlap the
                # second half's matmuls, so tT is never drain-gated.
                ups = [p0_ps.tile([128, SQ], F32, name=f"ups{j}", tag="ups", bufs=8)
                       for j in range(NJ)]

                def drain_u(j):
                    if apply_bq:
                        if j % 2 == 0:
                            nc.scalar.activation(uT[:, j, :], ups[j][:], AF.Identity,
                                                 bias=bq_sb[:, j:j + 1])
                        else:
                            nc.vector.tensor_scalar_add(uT[:, j, :], ups[j][:],
                                                        bq_sb[:, j:j + 1])
                    else:
                        if j % 2 == 0:
                            nc.scalar.activation(uT[:, j, :], ups[j][:], AF.Copy)
                        else:
                            nc.vector.tensor_copy(uT[:, j, :], ups[j][:])

                for t in range(NT):
                    for j in range(NJ):
                        nc.tensor.matmul(ups[j][:], wq_sb[:, t, 128 * j:128 * (j + 1)],
                                         qt_sb[:, t, :], start=(t == 0),
                                         stop=(t == NT - 1))
                for j in range(NJ):
                    drain_u(j)

                for j2 in range(NT):
                    ps = p0_ps.tile([128, SQ], F32, tag="ups", bufs=8)
                    for m in range(NJ):
                        nc.tensor.matmul(ps[:], wkt_sb[:, m, 128 * j2:128 * (j2 + 1)],
                                         uT[:, m, :], start=(m == 0), stop=(m == NJ - 1))
                    if j2 % 2 == 0:
                        nc.scalar.activation(tT[:, j2, :], ps[:], AF.Copy)
                    else:
                        nc.vector.tensor_copy(tT[:, j2, :], ps[:])
                p0_ps_ctx.__exit__(None, None, None)

            # cv PSUM pool opened BEFORE C0 so its banks are disjoint from the
            # score banks (else the first cv matmul WARs on the last exp read)
            cv_ctx = tc.tile_pool(name="cv_ps", bufs=3, space="PSUM")
            cv_ps_pool = cv_ctx.__enter__()

            # ---- C0: scoresT -> exp -> expT; DVE accumulates rowsum partials ----
            with (
                tc.tile_pool(name="sc_ps", bufs=3, space="PSUM") as sc_ps,
            ):
                kt_tiles = {0: kt_ch0}
                for c in range(NCH):
                    kt_ch = kt_tiles.pop(c)
                    # prefetch NEXT kt chunk before this chunk's v stream so
                    # the (single) DMA queue never starves the scores MMs
                    if c + 1 < NCH:
                        nxt = kt_pool.tile([128, NT, CH], BF16, tag="kt")
                        nc.sync.dma_start(
                            nxt[:],
                            ktf[:, CH * (c + 1):CH * (c + 2)].rearrange(
                                "(t p) s -> p t s", p=128))
                        kt_tiles[c + 1] = nxt
                    # v rides the SP queue AFTER kt[c+1]: FIFO keeps the head
                    # and the scores stream fed; v itself is only needed at CV.
                    # (On the ACT queue it would be dep-free and the scheduler
                    # hoists it to kernel start, starving the critical head.)
                    nc.sync.dma_start(
                        v_sb[:, 4 * c:4 * (c + 1), :],
                        vf[CH * c:CH * (c + 1), :].rearrange("(u p) d -> p u d", p=128))
                    for u in range(CH // 128):
                        idx = (CH // 128) * c + u
                        ps = sc_ps.tile([128, SQ], F32, tag="sps", bufs=3)
                        for t in range(NT):
                            nc.tensor.matmul(ps[:], kt_ch[:, t, 128 * u:128 * (u + 1)],
                                             tT[:, t, :], start=(t == 0), stop=(t == NT - 1))
                        nc.scalar.activation(expT[:, idx, :], ps[:], AF.Exp,
                                             scale=float(INV_SQRT_H))
                        if idx == 0:
                            nc.vector.tensor_copy(acc[:], expT[:, 0, :])
                        else:
                            nc.vector.tensor_tensor(acc[:], acc[:], expT[:, idx, :],
                                                    op=ALU.add)
            kt_ctx.__exit__(None, None, None)

            # ---- CV: cvT = (w @ v)^T accumulated directly; drain = normalize ----
            with (
                tc.tile_pool(name="wv_pool", bufs=1) as wv_pool,
                tc.tile_pool(name="pj_ps", bufs=2, space="PSUM") as pj_ps,
                tc.tile_pool(name="out_pool", bufs=2) as out_pool,
            ):
                wv_sb = wv_pool.tile([128, NT, H], F32R)
                for t in range(NT):
                    nc.sync.dma_start(wv_sb[:, t, :], wv[128 * t:128 * (t + 1), :])
                if apply_bv:
                    bv_bcast = pp.tile([128, H], F32)
                    with tc.tile_pool(name="bv_ps", bufs=2, space="PSUM") as bv_ps:
                        for half in range(2):
                            hs = slice(512 * half, 512 * (half + 1))
                            psb = bv_ps.tile([128, 512], F32)
                            nc.tensor.matmul(psb[:], onef[:], bv_row[0:1, hs],
                                             start=True, stop=True)
                            nc.scalar.activation(bv_bcast[:, hs], psb[:], AF.Copy)

                # rowsum partition-reduce entirely OFF the PE: GpSimd all-reduce
                # replicates sum_p acc[p, sq] across partitions (in-place),
                # then DVE takes the reciprocal.
                nc.gpsimd.partition_all_reduce(acc[:], acc[:], channels=128,
                                               reduce_op=bass_isa.ReduceOp.add)
                nc.vector.reciprocal(inv_bc[:], acc[:])
                for db in range(NT):
                    cvp = cv_ps_pool.tile([128, SQ], F32, tag="cvp", bufs=3)
                    for idx in range(NI):
                        nc.tensor.matmul(cvp[:], v_sb[:, idx, 128 * db:128 * (db + 1)],
                                         expT[:, idx, :], start=(idx == 0),
                                         stop=(idx == NI - 1))
                    # normalizing drain (PSUM f32 * inv -> SBUF f32r)
                    nc.vector.tensor_tensor(cvT[:, db, :], cvp[:], inv_bc[:],
                                            op=ALU.mult)

                # ---- PROJ: out = cvn @ Wv (+ bv) ----
                for b in range(SQ // 128):
                    for h_ in range(2):
                        hs = slice(512 * h_, 512 * (h_ + 1))
                        last = (b == SQ // 128 - 1 and h_ == 1)
                        if not last:
                            ps = pj_ps.tile([128, 512], F32, tag="ctx")
                            for t in range(NT):
                                nc.tensor.matmul(ps[:], cvT[:, t, 128 * b:128 * (b + 1)],
                                                 wv_sb[:, t, hs], start=(t == 0),
                                                 stop=(t == NT - 1))
                            out_t = out_pool.tile([128, 512], F32, tag="out")
                            if apply_bv:
                                nc.vector.tensor_tensor(out_t[:], ps[:], bv_bcast[:, hs],
                                                        op=ALU.add)
                            else:
                                nc.scalar.activation(out_t[:], ps[:], AF.Copy)
                            nc.sync.dma_start(out[128 * b:128 * (b + 1), hs], out_t[:])
                        else:
                            # final tile: two half-N banks so the two drains +
                            # DMAs run in parallel on ACT and DVE
                            out_t = out_pool.tile([128, 512], F32, tag="out")
                            for qh in range(2):
                                qcol = slice(512 * h_ + 256 * qh,
                                             512 * h_ + 256 * (qh + 1))
                                qs = slice(256 * qh, 256 * (qh + 1))
                                psq = pj_ps.tile([128, 256], F32, tag=f"ctxl{qh}",
                                                 bufs=1)
                                for t in range(NT):
                                    nc.tensor.matmul(
                                        psq[:], cvT[:, t, 128 * b:128 * (b + 1)],
                                        wv_sb[:, t, qcol], start=(t == 0),
                                        stop=(t == NT - 1))
                                if apply_bv:
                                    nc.vector.tensor_tensor(out_t[:, qs], psq[:],
                                                            bv_bcast[:, qcol],
                                                            op=ALU.add)
                                    nc.sync.dma_start(
                                        out[128 * b:128 * (b + 1), qcol], out_t[:, qs])
                                elif qh == 0:
                                    nc.scalar.activation(out_t[:, qs], psq[:], AF.Copy)
                                    nc.scalar.dma_start(
                                        out[128 * b:128 * (b + 1), qcol], out_t[:, qs])
                                else:
                                    nc.vector.tensor_copy(out_t[:, qs], psq[:])
                                    nc.sync.dma_start(
                                        out[128 * b:128 * (b + 1), qcol], out_t[:, qs])
            cv_ctx.__exit__(None, None, None)

    nc.compile()
    return nc


_CACHE = {}


def _get_program(apply_bq: bool, apply_bv: bool):
    key = (apply_bq, apply_bv)
    if key not in _CACHE:
        _CACHE[key] = build_program(apply_bq, apply_bv)
    return _CACHE[key]


def _prepare_in_maps(ins: dict) -> list:
    import ml_dtypes
    q = np.asarray(ins["q"], np.float32)
    k = np.asarray(ins["k"], np.float32)
    v = np.asarray(ins["v"], np.float32)
    assert q.shape == (S, D) and k.shape == (S, D) and v.shape == (S, D)

    qT = np.ascontiguousarray(q.T).astype(ml_dtypes.bfloat16)
    kT_bf = np.ascontiguousarray(k.T).astype(ml_dtypes.bfloat16)
    v_bf = v.astype(ml_dtypes.bfloat16)
    Wq = np.ascontiguousarray(np.asarray(ins["Wq"], np.float32)).astype(ml_dtypes.bfloat16)
    WkT = np.ascontiguousarray(np.asarray(ins["Wk"], np.float32).T).astype(ml_dtypes.bfloat16)
    Wv = np.ascontiguousarray(np.asarray(ins["Wv"], np.float32))
    bq = np.asarray(ins["bq"], np.float32).reshape(H)
    bv = np.asarray(ins["bv"], np.float32).reshape(H)

    bq_r = np.ascontiguousarray(bq.reshape(NJ, 128))
    bv_d = np.ascontiguousarray(bv.reshape(1, H))

    in_maps = []
    for i in range(NCORES):
        sl = slice(SQ * i, SQ * (i + 1))
        in_maps.append({
            "qt": np.ascontiguousarray(qT[:, sl]),
            "ktf": kT_bf, "vf": v_bf,
            "wq": Wq, "wkt": WkT, "wv": Wv,
            "ones_d": np.ones((128, 128), np.float32),
            "ones_bb": np.ones((128, 128), ml_dtypes.bfloat16),
            "bq_r": bq_r, "bv_d": bv_d,
            "ones_f": np.ones((1, 128), np.float32),
        })
    return in_maps


def kernel(q, k, v, Wq, bq, Wk, bk, Wv, bv) -> np.ndarray:
    # bk shifts all scores in a row uniformly and cancels in softmax.
    ins = {"q": q, "k": k, "v": v, "Wq": Wq, "bq": bq, "Wk": Wk,
           "Wv": Wv, "bv": bv}
    apply_bq = bool(np.any(np.asarray(bq)))
    apply_bv = bool(np.any(np.asarray(bv)))
    nc = _get_program(apply_bq, apply_bv)
    in_maps = _prepare_in_maps(ins)
    res = run_bass_kernel_spmd(nc, in_maps, core_ids=list(range(NCORES)))
    return np.concatenate([res.results[i]["out"] for i in range(NCORES)], axis=0)
